# revision 1
# baseline (speedup 1.0000x reference)
"""Trainium2 Bass kernel for nn_FGNet (gnn_message_passing).

Strategy
--------
Per-edge weights are gathers from tiny tables (169 edge types), so edges are
sorted by type id and processed in uniform 256-edge blocks (one id per block,
padded; 2 segments x 128 edges).  Device math per block:

    t_h   = relu(W_id.T @ feats_h + b_id)        h = 0,1
    p_h,i = prod_{j != i} t_h,j                  products, 3 wide DVE muls
    msg_i = W2_id,i.T @ [p_0,i | p_1,i]          N=256 matmul per i
    (the second bias b2 is linear in the segment-sum -> folded to the host)

Matmuls run in float32r (single-pass fp32, ~1.5e-4 relmax, 4x faster than
fp32's 2-pass mode).  HW constraints found empirically on this stack:
  - f32r matmuls need K=128 (K=64 silently returns zeros)
  - matmul *input* partition offsets crash the runtime (NRT unrecoverable)
  - f32r + nonzero *output* partition offset emits tile_position -> invalid ISA
  - DVE memset of an f32r AP is invalid ISA (memset via an f32 bitcast)
  - every instruction gets at most ONE sync wait; Bacc.finalize()'s
    generate_event_semaphores pass splits multi-waits legally
So the transform runs K=128 with zero-padded stationary weights [W;0]/[0;W]
(zeros baked host-side into the packed block), and the second matmul keeps
all outputs at partition offset 0: per block ps2 is [64, 3, 256]; block pairs
are merged into a [128, 768] tile (GPSIMD does the cross-partition move for
odd blocks) so the store DMA uses all 128 partitions / 16 DMA ports.

Packed input per block (pk: [128 partitions, 832 f32r columns]):
    cols   0:384  feats   p = 64*h + l, col = i*128 + e
    cols 384:512  wA = [W; 0]
    cols 512:640  wB = [0; W]
    cols 640:832  ho      row r, col i*64 + l = ho_params[i, id, r, l]
Output msgs[q, 64*parity + e, i*256 + h*128... ] -- see _postprocess.

Host side (vectorized numpy): id computation, sort, feature gather, packing,
unpermute, b2 bias add and the final segment-sum into node_msg.
"""

import numpy as np

_BLK = 256          # edge slots per block (2 segments x 128)
_SEG = 128
_FCOLS = 832        # packed pk columns per block
_NCORES = 8

_prog_cache = {}


def _build_program(B):
    """Build the SPMD device program for B blocks per core (B even)."""
    import concourse.mybir as mybir
    import concourse.tile as tile
    from concourse import bacc

    F32 = mybir.dt.float32
    F32R = mybir.dt.float32r
    Relu = mybir.ActivationFunctionType.Relu
    Copy = mybir.ActivationFunctionType.Copy

    assert B % 2 == 0
    PB = B // 2

    nc = bacc.Bacc()
    pk = nc.declare_dram_parameter("pk", [B, 128, _FCOLS], F32R, isOutput=False)
    bia = nc.declare_dram_parameter("bia", [128, B], F32, isOutput=False)
    msgs = nc.declare_dram_parameter("msgs", [B, 64, 768], F32, isOutput=True)

    with tile.TileContext(nc) as tc:
        with (
            tc.tile_pool(name="const", bufs=1) as const,
            tc.tile_pool(name="work", bufs=4) as work,
            tc.tile_pool(name="psum", bufs=2, space="PSUM") as psum,
        ):
            bt = const.tile([128, B], F32, name="bt")
            nc.sync.dma_start(out=bt[:], in_=bia[:])

            for b in range(B):
                pkt = work.tile([128, _FCOLS], F32R, name="pkt", tag="pkt")
                nc.sync.dma_start(out=pkt[:], in_=pk[b])

                # transform: one psum tile, both segments
                ps1 = psum.tile([128, 2, 512], F32, name="ps1", tag="ps1")
                nc.tensor.matmul(out=ps1[:, 0, 0:384], lhsT=pkt[:, 384:512],
                                 rhs=pkt[:, 0:384], start=True, stop=True)
                nc.tensor.matmul(out=ps1[:, 1, 0:384], lhsT=pkt[:, 512:640],
                                 rhs=pkt[:, 0:384], start=True, stop=True)

                t = work.tile([128, 2, 384], F32, name="t", tag="t")
                nc.scalar.activation(out=t[:], in_=ps1[:, :, 0:384],
                                     func=Relu, bias=bt[:, b:b + 1],
                                     scale=1.0)

                # products: p[:, i, h, :] = prod_{j != i} t_h,j
                p = work.tile([128, 3, 2, 128], F32R, name="p", tag="p")
                nc.vector.tensor_mul(out=p[:, 0], in0=t[:, :, 128:256],
                                     in1=t[:, :, 256:384])
                nc.vector.tensor_mul(out=p[:, 1], in0=t[:, :, 0:128],
                                     in1=t[:, :, 256:384])
                nc.vector.tensor_mul(out=p[:, 2], in0=t[:, :, 0:128],
                                     in1=t[:, :, 128:256])

                # second matmul: msg_i = ho_i.T @ [p_0,i | p_1,i], N=256
                ps2 = psum.tile([64, 3, 256], F32, name="ps2", tag="ps2")
                for i in range(3):
                    nc.tensor.matmul(
                        out=ps2[:, i, :],
                        lhsT=pkt[:, 640 + 64 * i:640 + 64 * (i + 1)],
                        rhs=p[:, i].rearrange("r h e -> r (h e)"),
                        start=True, stop=True,
                    )

                ps2f = ps2[:].rearrange("l i he -> l (i he)")
                m = work.tile([64, 768], F32, name="m", tag="m")
                if b % 2 == 0:
                    nc.vector.tensor_copy(out=m[:], in_=ps2f)
                else:
                    nc.scalar.activation(out=m[:], in_=ps2f, func=Copy,
                                         bias=0.0, scale=1.0)
                nc.sync.dma_start(out=msgs[b], in_=m[:])
    nc.finalize()
    return nc


def _get_program(B):
    if B not in _prog_cache:
        _prog_cache[B] = _build_program(B)
    return _prog_cache[B]


def _prepare(x, nodes, fact, params, bias_p, ho_params, ho_bias):
    """Host-side: sort by id, build per-block packed arrays."""
    N, L = nodes.shape
    E = fact.shape[0]
    R = params.shape[2]
    NP = params.shape[0]           # 169
    MA = int(round(NP ** 0.5))     # 13

    ids = (x[fact[:, 0], 1] * MA + x[fact[:, 0], 2]).astype(np.int64)   # [E]
    perm = np.argsort(ids, kind="stable")
    ids_s = ids[perm]
    fact_s = fact[perm].astype(np.int64)                                 # [E,3]

    counts = np.bincount(ids_s, minlength=NP)                            # [NP]
    nblk = (counts + _BLK - 1) // _BLK                                   # [NP]
    blk_ids = np.repeat(np.arange(NP), nblk)                             # [NB]
    NB = int(blk_ids.shape[0])
    B = (NB + _NCORES - 1) // _NCORES
    if B % 2:
        B += 1
    NB8 = B * _NCORES
    blk_ids = np.concatenate([blk_ids, np.zeros(NB8 - NB, np.int64)])

    # slot -> sorted-edge-position map (-1 = padding)
    padded = nblk * _BLK
    pad_off = np.concatenate([[0], np.cumsum(padded)])
    off = np.concatenate([[0], np.cumsum(counts)])
    total = int(pad_off[-1])
    t_of = np.repeat(np.arange(NP), padded)
    jloc = np.arange(total) - pad_off[t_of]
    src = np.where(jloc < counts[t_of], off[t_of] + jloc, -1)
    src = np.concatenate([src, np.full(NB8 * _BLK - total, -1, np.int64)])
    valid = src >= 0

    # gather features per slot
    nf = nodes[fact_s]                                                   # [E,3,L]
    featp = np.zeros((NB8 * _BLK, 3, L), np.float32)
    featp[valid] = nf[src[valid]]

    # pack feats + [W;0] + [0;W] + ho
    pk = np.zeros((NB8, 128, _FCOLS), np.float32)
    pk[:, :, 0:384] = (
        featp.reshape(NB8, 2, _SEG, 3, L).transpose(0, 1, 4, 3, 2)
        .reshape(NB8, 128, 384)
    )
    W = params[blk_ids].astype(np.float32)                               # [NB8,L,R]
    pk[:, 0:64, 384:512] = W
    pk[:, 64:128, 512:640] = W
    pk[:, :, 640:832] = (
        ho_params[:, blk_ids].astype(np.float32).transpose(1, 2, 0, 3)
        .reshape(NB8, R, 3 * L)
    )

    biasT = bias_p[blk_ids, 0].astype(np.float32)                        # [NB8,R]
    biasT = biasT.reshape(_NCORES, B, R).transpose(0, 2, 1)              # [8,R,B]

    return dict(pk=pk, biasT=np.ascontiguousarray(biasT), B=B, NB8=NB8,
                src=src, valid=valid, fact_s=fact_s, ids_s=ids_s,
                N=N, E=E, L=L)


def _postprocess(msgs_all, prep, ho_bias):
    """Decode per-slot messages, add host-side b2, segment-sum into node_msg."""
    NB8, N, E, L = prep["NB8"], prep["N"], prep["E"], prep["L"]
    src, valid, fact_s, ids_s = prep["src"], prep["valid"], prep["fact_s"], prep["ids_s"]
    # msgs_all [NB8, 64, 768]: row = l, col = i*256 + h*128 + e
    slots = (
        msgs_all.reshape(NB8, 64, 3, 2, _SEG).transpose(0, 3, 4, 2, 1)
        .reshape(NB8 * _BLK, 3, 64)
    )
    msg_e = np.empty((E, 3, L), np.float32)
    msg_e[src[valid]] = slots[valid]

    # fold in the second bias (linear in the segment-sum)
    msg_e += ho_bias[:, ids_s, 0].astype(np.float32).transpose(1, 0, 2)  # [E,3,L]

    idx_all = fact_s.T.reshape(-1)                                       # [3E]
    val_all = msg_e.transpose(1, 0, 2).reshape(-1, L)                    # [3E,L]
    order = np.argsort(idx_all, kind="stable")
    idx_sorted = idx_all[order]
    val_sorted = val_all[order]
    uniq, starts = np.unique(idx_sorted, return_index=True)
    sums = np.add.reduceat(val_sorted, starts, axis=0)
    out = np.zeros((N, L), np.float32)
    out[uniq] = sums
    return out


def _run_device(prep, trace=False, trace_kwargs=None):
    from concourse.bass_utils import run_bass_kernel_spmd

    B = prep["B"]
    nc = _get_program(B)
    in_maps = []
    for c in range(_NCORES):
        in_maps.append({
            "pk": prep["pk"][c * B:(c + 1) * B],
            "bia": prep["biasT"][c],
        })
    kwargs = {}
    if trace:
        kwargs["trace"] = True
        if trace_kwargs:
            kwargs.update(trace_kwargs)
    res = run_bass_kernel_spmd(nc, in_maps, list(range(_NCORES)), **kwargs)
    msgs_all = np.concatenate([res.results[c]["msgs"] for c in range(_NCORES)],
                              axis=0)
    return msgs_all, res


def kernel(x, nodes, fact, fact_dim, params, bias_p, ho_params, ho_bias,
           _trace=False, _trace_kwargs=None):
    x = np.asarray(x)
    nodes = np.asarray(nodes, dtype=np.float32)
    fact = np.asarray(fact)
    params = np.asarray(params)
    bias_p = np.asarray(bias_p)
    ho_params = np.asarray(ho_params)
    ho_bias = np.asarray(ho_bias)

    prep = _prepare(x, nodes, fact, params, bias_p, ho_params, ho_bias)
    msgs_all, res = _run_device(prep, trace=_trace, trace_kwargs=_trace_kwargs)
    out = _postprocess(msgs_all, prep, ho_bias)
    kernel.last_results = res
    return out



# revision 5
# speedup vs baseline: 1.5145x; 1.5145x over previous
"""Trainium2 Bass kernel for nn_FGNet (gnn_message_passing).

Strategy (v3)
-------------
Per-edge weights are gathers from tiny tables (169 edge types), so edges are
sorted by type id and processed in uniform 256-edge blocks (one id per block,
padded; 2 segments x 128 edges).  Device math per block, all bf16 in / f32
accumulate:

    t_h   = relu(W_id.T @ feats_h + b_id)        h = 0,1   (K=64 matmul)
    p_h,i = prod_{j != i} t_h,j                  3 DVE muls, bf16
    msg_i = ho_id,i.T @ [p_0,i | p_1,i]          K=128, N=256 matmul
    ps2 -> m (bf16) copy on gpsimd/vector/scalar round-robin, DMA out
    (the second bias b2 is linear in the segment-sum -> folded on the host)

bf16 matmuls run at 1 col/cycle at full PE p-state and halve all DMA bytes
vs the f32r baseline (end-to-end rel err ~2e-3, tolerance 2e-2).  The ho
tables for all B blocks are hoisted into SBUF once ([128, B*192] bf16), W
streams with the feats (one [64, 2, 896] DMA per block pair), outputs are
written bf16 one DMA per block pair.  Empirical HW constraints inherited
from the baseline: matmul *input* partition offsets crash the runtime, so
all matmul operands start at partition 0; PSUM cannot be DMA'd directly, so
MM2 results are copied (and converted) to SBUF first.

Packed layouts per core (B blocks):
    fz   [B//2, 64, 2, 896] bf16   feats col h*384 + i*128 + e (l=partition),
                                   then W_id (lhsT [64, 128]) in cols 768:896
    hot  [128, B*192] bf16         col b*192 + i*64 + l, partition r
    bia  [128, B] f32              bias_p[id].T per block
    msgs [B//2, 64, 2, 768] bf16   row l, col i*256 + h*128 + e

Host side (vectorized numpy): id computation, sort, feature gather, packing,
unpermute, b2 bias add and the final segment-sum into node_msg.
"""

import numpy as np

_BLK = 256          # edge slots per block (2 segments x 128)
_SEG = 128
_NCORES = 8

_prog_cache = {}


def _build_program(B):
    """Build the SPMD device program for B blocks per core (B even)."""
    import concourse.mybir as mybir
    import concourse.tile as tile
    from concourse import bacc

    F32 = mybir.dt.float32
    BF16 = mybir.dt.bfloat16
    Relu = mybir.ActivationFunctionType.Relu

    assert B % 2 == 0

    nc = bacc.Bacc()
    fz = nc.declare_dram_parameter("fz", [B // 2, 64, 2, 896], BF16,
                                   isOutput=False)
    hot = nc.declare_dram_parameter("hot", [128, B * 192], BF16,
                                    isOutput=False)
    bia = nc.declare_dram_parameter("bia", [128, B], F32, isOutput=False)
    msgs = nc.declare_dram_parameter("msgs", [B // 2, 64, 2, 768], BF16,
                                     isOutput=True)

    with tile.TileContext(nc) as tc:
        with (
            tc.tile_pool(name="const", bufs=1) as const,
            tc.tile_pool(name="work", bufs=4) as work,
            tc.tile_pool(name="outp", bufs=3) as outp,
            tc.tile_pool(name="psum", bufs=2, space="PSUM") as psum,
        ):
            bt = const.tile([128, B], F32, name="bt")
            nc.sync.dma_start(out=bt[:], in_=bia[:])
            ht = const.tile([128, B * 192], BF16, name="ht")
            # split the big table load so block 0 can start early
            nsplit = 4
            step = (B + nsplit - 1) // nsplit
            for s in range(0, B, step):
                e = min(s + step, B)
                nc.sync.dma_start(out=ht[:, s * 192:e * 192],
                                  in_=hot[:, s * 192:e * 192])

            ncopy = 0
            for q in range(B // 2):
                fk = work.tile([64, 2, 896], BF16, name="fk", tag="fk")
                nc.sync.dma_start(out=fk[:], in_=fz[q])
                m = outp.tile([64, 2, 768], BF16, name="m", tag="m")
                for g in range(2):
                    b = 2 * q + g
                    ps1 = psum.tile([128, 2, 512], F32, name="ps1", tag="ps1")
                    for h in range(2):
                        nc.tensor.matmul(out=ps1[:, h, 0:384],
                                         lhsT=fk[:, g, 768:896],
                                         rhs=fk[:, g, h * 384:(h + 1) * 384],
                                         start=True, stop=True)
                    t = work.tile([128, 2, 384], BF16, name="t", tag="t")
                    nc.scalar.activation(out=t[:], in_=ps1[:, :, 0:384],
                                         func=Relu, bias=bt[:, b:b + 1],
                                         scale=1.0)

                    p = work.tile([128, 3, 2, 128], BF16, name="p", tag="p")
                    for i, (j, k) in enumerate(((1, 2), (0, 2), (0, 1))):
                        nc.vector.tensor_mul(
                            out=p[:, i],
                            in0=t[:, :, 128 * j:128 * (j + 1)],
                            in1=t[:, :, 128 * k:128 * (k + 1)])

                    ps2 = psum.tile([64, 3, 256], F32, name="ps2", tag="ps2")
                    for i in range(3):
                        nc.tensor.matmul(
                            out=ps2[:, i, :],
                            lhsT=ht[:, b * 192 + i * 64:b * 192 + (i + 1) * 64],
                            rhs=p[:, i].rearrange("r h e -> r (h e)"),
                            start=True, stop=True)

                    # GPSIMD cannot read PSUM on TRN2: split the psum->sbuf
                    # copies between DVE and the scalar engine (3:2).
                    src = ps2[:].rearrange("l i he -> l (i he)")
                    if ncopy % 5 in (1, 4):
                        nc.scalar.copy(out=m[:, g, :], in_=src)
                    else:
                        nc.vector.tensor_copy(out=m[:, g, :], in_=src)
                    ncopy += 1
                nc.sync.dma_start(out=msgs[q], in_=m[:])
    nc.finalize()
    return nc


def _get_program(B):
    if B not in _prog_cache:
        _prog_cache[B] = _build_program(B)
    return _prog_cache[B]


def _prepare(x, nodes, fact, params, bias_p, ho_params, ho_bias):
    """Host-side: sort by id, build per-block packed arrays."""
    import ml_dtypes
    bf16 = ml_dtypes.bfloat16

    N, L = nodes.shape
    E = fact.shape[0]
    R = params.shape[2]
    NP = params.shape[0]           # 169
    MA = int(round(NP ** 0.5))     # 13

    ids = (x[fact[:, 0], 1] * MA + x[fact[:, 0], 2]).astype(np.int64)   # [E]
    perm = np.argsort(ids, kind="stable")
    ids_s = ids[perm]
    fact_s = fact[perm].astype(np.int64)                                 # [E,3]

    counts = np.bincount(ids_s, minlength=NP)                            # [NP]
    nblk = (counts + _BLK - 1) // _BLK                                   # [NP]
    blk_ids = np.repeat(np.arange(NP), nblk)                             # [NB]
    NB = int(blk_ids.shape[0])
    B = (NB + _NCORES - 1) // _NCORES
    if B % 2:
        B += 1
    NB8 = B * _NCORES
    blk_ids = np.concatenate([blk_ids, np.zeros(NB8 - NB, np.int64)])

    # slot -> sorted-edge-position map (-1 = padding)
    padded = nblk * _BLK
    pad_off = np.concatenate([[0], np.cumsum(padded)])
    off = np.concatenate([[0], np.cumsum(counts)])
    total = int(pad_off[-1])
    t_of = np.repeat(np.arange(NP), padded)
    jloc = np.arange(total) - pad_off[t_of]
    src = np.where(jloc < counts[t_of], off[t_of] + jloc, -1)
    src = np.concatenate([src, np.full(NB8 * _BLK - total, -1, np.int64)])
    valid = src >= 0

    # gather features per slot
    nf = nodes[fact_s].astype(bf16)                                      # [E,3,L]
    featp = np.zeros((NB8 * _BLK, 3, L), bf16)
    featp[valid] = nf[src[valid]]

    # fz: feats (cols h*384 + i*128 + e over partitions l) + W (cols 768:896)
    fz = np.zeros((NB8, 64, 896), bf16)
    fz[:, :, 0:768] = (
        featp.reshape(NB8, 2, _SEG, 3, L).transpose(0, 4, 1, 3, 2)
        .reshape(NB8, 64, 768)
    )
    fz[:, :, 768:896] = params[blk_ids].astype(bf16)                     # [NB8,L,R]
    fz = fz.reshape(_NCORES, B // 2, 2, 64, 896).transpose(0, 1, 3, 2, 4)
    fz = np.ascontiguousarray(fz)                                        # [8,B/2,64,2,896]

    # hot: [128, B*192], col b*192 + i*64 + l
    hot = (
        ho_params[:, blk_ids].astype(bf16).transpose(1, 2, 0, 3)
        .reshape(NB8, R, 3 * L)
    )                                                                    # [NB8,128,192]
    hot = hot.reshape(_NCORES, B, R, 3 * L).transpose(0, 2, 1, 3)
    hot = np.ascontiguousarray(hot.reshape(_NCORES, R, B * 3 * L))       # [8,128,B*192]

    biasT = bias_p[blk_ids, 0].astype(np.float32)                        # [NB8,R]
    biasT = biasT.reshape(_NCORES, B, R).transpose(0, 2, 1)              # [8,R,B]

    return dict(fz=fz, hot=hot, biasT=np.ascontiguousarray(biasT), B=B,
                NB8=NB8, src=src, valid=valid, fact_s=fact_s, ids_s=ids_s,
                N=N, E=E, L=L)


def _postprocess(msgs_all, prep, ho_bias):
    """Decode per-slot messages, add host-side b2, segment-sum into node_msg."""
    NB8, N, E, L = prep["NB8"], prep["N"], prep["E"], prep["L"]
    src, valid, fact_s, ids_s = prep["src"], prep["valid"], prep["fact_s"], prep["ids_s"]
    # msgs_all [NB8, 64, 768] f32: row = l, col = i*256 + h*128 + e
    slots = (
        msgs_all.reshape(NB8, 64, 3, 2, _SEG).transpose(0, 3, 4, 2, 1)
        .reshape(NB8 * _BLK, 3, 64)
    )
    msg_e = np.empty((E, 3, L), np.float32)
    msg_e[src[valid]] = slots[valid]

    # fold in the second bias (linear in the segment-sum)
    msg_e += ho_bias[:, ids_s, 0].astype(np.float32).transpose(1, 0, 2)  # [E,3,L]

    idx_all = fact_s.T.reshape(-1)                                       # [3E]
    val_all = msg_e.transpose(1, 0, 2).reshape(-1, L)                    # [3E,L]
    order = np.argsort(idx_all, kind="stable")
    idx_sorted = idx_all[order]
    val_sorted = val_all[order]
    uniq, starts = np.unique(idx_sorted, return_index=True)
    sums = np.add.reduceat(val_sorted, starts, axis=0)
    out = np.zeros((N, L), np.float32)
    out[uniq] = sums
    return out


def _run_device(prep, trace=False, trace_kwargs=None):
    from concourse.bass_utils import run_bass_kernel_spmd

    B = prep["B"]
    nc = _get_program(B)
    in_maps = []
    for c in range(_NCORES):
        in_maps.append({
            "fz": prep["fz"][c],
            "hot": prep["hot"][c],
            "bia": prep["biasT"][c],
        })
    kwargs = {}
    if trace:
        kwargs["trace"] = True
        if trace_kwargs:
            kwargs.update(trace_kwargs)
    res = run_bass_kernel_spmd(nc, in_maps, list(range(_NCORES)), **kwargs)
    msgs_all = np.concatenate(
        [np.asarray(res.results[c]["msgs"]).astype(np.float32)
         .transpose(0, 2, 1, 3).reshape(-1, 64, 768)
         for c in range(_NCORES)], axis=0)
    return msgs_all, res


def kernel(x, nodes, fact, fact_dim, params, bias_p, ho_params, ho_bias,
           _trace=False, _trace_kwargs=None):
    x = np.asarray(x)
    nodes = np.asarray(nodes, dtype=np.float32)
    fact = np.asarray(fact)
    params = np.asarray(params)
    bias_p = np.asarray(bias_p)
    ho_params = np.asarray(ho_params)
    ho_bias = np.asarray(ho_bias)

    prep = _prepare(x, nodes, fact, params, bias_p, ho_params, ho_bias)
    msgs_all, res = _run_device(prep, trace=_trace, trace_kwargs=_trace_kwargs)
    out = _postprocess(msgs_all, prep, ho_bias)
    kernel.last_results = res
    return out
